# revision 1
# baseline (speedup 1.0000x reference)
"""Trainium2 Bass kernel for LowRankOrthogonalProjection.

Reference computes:
    P = W @ W.T                      (D,D) projection onto rank-R subspace
    C = I - P
    out = target @ C.T + source @ P.T

Since P is symmetric and idempotent-free in the formula, this is exactly
    out = target + (source - target) @ W @ W.T
which replaces two (B*S,D)x(D,D) dense matmuls (~0.55 PFLOP) with two
rank-16 matmuls (~2 GFLOP).  The kernel is therefore memory-bound.

Sharding: data-parallel over the flattened (B*S) row dimension across 8
cores; W (tiny) replicated.  No cross-core communication.

Per-core dataflow (rows-per-core RPC = 2048, D = 4096):
  for each 128-row block:
    DMA src (bf16, host-cast) and tgt (f32) tiles in
    DVE:  diff = src - tgt                  -> bf16
    PE :  transpose 128x128 chunks of diff  -> PSUM (bf16)
    ACT:  copy PSUM -> SBUF (diffT)
    PE :  tmpT(16,128)  += Wchunk.T @ diffT  (accumulate over 32 D-chunks)
    ACT:  tmpT -> SBUF bf16
    PE :  corr(128,512) = tmpT.T @ W.T       (8 chunks)
    DVE:  out = corr + tgt                   (f32)
    DMA out
"""

import numpy as np
import ml_dtypes

B, S, D, R = 4, 4096, 4096, 16
N_CORES = 8
ROWS = B * S                 # 16384
RPC = ROWS // N_CORES        # 2048 rows per core
P = 128
DCH = D // P                 # 32 D-chunks

_NC_CACHE = {}


def build_nc(rpc=RPC, reps=1, io_bufs=3, out_bufs=2, out_dma_scalar=False):
    import concourse.bass as bass
    import concourse.mybir as mybir
    import concourse.tile as tile

    bf16 = mybir.dt.bfloat16
    f32 = mybir.dt.float32

    nc = bass.Bass("TRN2", target_bir_lowering=False)

    src = nc.dram_tensor("src", [rpc, D], bf16, kind="ExternalInput")
    tgt = nc.dram_tensor("tgt", [rpc, D], f32, kind="ExternalInput")
    # wc[p, t*R + r] = W[t*128 + p, r]   (W chunked along D for lhsT use)
    wc = nc.dram_tensor("wc", [P, DCH * R], bf16, kind="ExternalInput")
    # wt[r, d] = W[d, r]
    wt = nc.dram_tensor("wt", [R, D], bf16, kind="ExternalInput")
    ident = nc.dram_tensor("ident", [P, P], bf16, kind="ExternalInput")
    out = nc.dram_tensor("out", [rpc, D], f32, kind="ExternalOutput")

    nblk = rpc // P

    with tile.TileContext(nc) as tc:
        with (
            tc.tile_pool(name="const", bufs=1) as cpool,
            tc.tile_pool(name="srcp", bufs=io_bufs) as src_pool,
            tc.tile_pool(name="tgtp", bufs=io_bufs) as tgt_pool,
            tc.tile_pool(name="diffp", bufs=2) as diff_pool,
            tc.tile_pool(name="dtp", bufs=3) as dt_pool,
            tc.tile_pool(name="tmtp", bufs=2) as tmt_pool,
            tc.tile_pool(name="outp", bufs=out_bufs) as out_pool,
            tc.tile_pool(name="ps_t", bufs=3, space="PSUM") as ps_t,
            tc.tile_pool(name="ps_acc", bufs=2, space="PSUM") as ps_acc,
            tc.tile_pool(name="ps_out", bufs=3, space="PSUM") as ps_out,
        ):
            wc_sb = cpool.tile([P, DCH * R], bf16)
            nc.sync.dma_start(wc_sb, wc[:, :])
            wt_sb = cpool.tile([R, D], bf16)
            nc.sync.dma_start(wt_sb, wt[:, :])
            id_sb = cpool.tile([P, P], bf16)
            nc.sync.dma_start(id_sb, ident[:, :])

            for rb in range(nblk * reps):
                rb = rb % nblk
                rs = rb * P
                src_sb = src_pool.tile([P, D], bf16, tag="src")
                tgt_sb = tgt_pool.tile([P, D], f32, tag="tgt")
                nc.sync.dma_start(src_sb, src[rs : rs + P, :])
                nc.sync.dma_start(tgt_sb, tgt[rs : rs + P, :])

                diff_sb = diff_pool.tile([P, D], bf16, tag="diff")
                nc.vector.tensor_sub(diff_sb, src_sb, tgt_sb)

                tmpT_ps = ps_acc.tile([R, P], f32, tag="tmtps")
                for g in range(DCH // 4):
                    tp = ps_t.tile([P, 4 * P], bf16, tag="tps")
                    for j in range(4):
                        t = g * 4 + j
                        nc.tensor.transpose(
                            tp[:, j * P : (j + 1) * P],
                            diff_sb[:, t * P : (t + 1) * P],
                            id_sb,
                        )
                    dT_sb = dt_pool.tile([P, 4 * P], bf16, tag="dT")
                    nc.scalar.copy(dT_sb, tp)
                    for j in range(4):
                        t = g * 4 + j
                        nc.tensor.matmul(
                            tmpT_ps,
                            wc_sb[:, t * R : (t + 1) * R],
                            dT_sb[:, j * P : (j + 1) * P],
                            start=(t == 0),
                            stop=(t == DCH - 1),
                        )

                tmpT_sb = tmt_pool.tile([R, P], bf16, tag="tmt")
                nc.scalar.copy(tmpT_sb, tmpT_ps)

                out_sb = out_pool.tile([P, D], f32, tag="out")
                for g in range(D // 512):
                    op = ps_out.tile([P, 512], f32, tag="ops")
                    nc.tensor.matmul(
                        op,
                        tmpT_sb,
                        wt_sb[:, g * 512 : (g + 1) * 512],
                        start=True,
                        stop=True,
                    )
                    nc.vector.tensor_add(
                        out_sb[:, g * 512 : (g + 1) * 512],
                        op,
                        tgt_sb[:, g * 512 : (g + 1) * 512],
                    )
                out_eng = nc.scalar if out_dma_scalar else nc.sync
                out_eng.dma_start(out[rs : rs + P, :], out_sb)

    return nc


def split_waits(nc, limit=1):
    """Walrus in this toolchain encodes at most one semaphore wait per
    instruction ("Too many sync wait commands").  Tile's scheduler attaches
    up to ~3.  Rewrite: keep the last wait on the instruction and hoist the
    rest onto standalone EventSemaphore instructions (same engine, placed
    immediately before), which is exactly what raw-bass wait_ge emits."""
    import concourse.mybir as mybir

    nsplit = 0
    for fn in nc.m.functions:
        for blk in fn.blocks:
            new = []
            for ins in blk.instructions:
                si = ins.sync_info
                waits = list(si.on_wait) if si is not None and si.on_wait else []
                if len(waits) > limit:
                    for k, w in enumerate(waits[:-limit]):
                        es = mybir.InstEventSemaphore(
                            name=f"{ins.name}-hw{k}",
                            engine=ins.engine,
                            sync_info=mybir.SyncInfo(on_wait=[w], on_update=[]),
                        )
                        new.append(es)
                        nsplit += 1
                    ins.sync_info = mybir.SyncInfo(
                        on_wait=waits[-limit:],
                        on_update=list(si.on_update or []),
                    )
                new.append(ins)
            blk.instructions[:] = new
    return nsplit


def _get_nc(rpc=RPC, reps=1):
    key = (rpc, reps)
    if key not in _NC_CACHE:
        nc = build_nc(rpc, reps)
        nc.finalize()
        split_waits(nc)
        _NC_CACHE[key] = nc
    return _NC_CACHE[key]


def make_host_inputs(source, target, weight):
    """Cast/shard host-side; returns per-core in_maps."""
    bf = ml_dtypes.bfloat16
    src2 = np.ascontiguousarray(source.reshape(ROWS, D)).astype(bf)
    tgt2 = np.ascontiguousarray(target.reshape(ROWS, D))
    if tgt2.dtype != np.float32:
        tgt2 = tgt2.astype(np.float32)
    wc = np.ascontiguousarray(
        weight.reshape(DCH, P, R).transpose(1, 0, 2).reshape(P, DCH * R)
    ).astype(bf)
    wt = np.ascontiguousarray(weight.T).astype(bf)
    ident = np.eye(P, dtype=np.float32).astype(bf)
    in_maps = []
    for c in range(N_CORES):
        sl = slice(c * RPC, (c + 1) * RPC)
        in_maps.append(
            {
                "src": np.ascontiguousarray(src2[sl]),
                "tgt": np.ascontiguousarray(tgt2[sl]),
                "wc": wc,
                "wt": wt,
                "ident": ident,
            }
        )
    return in_maps


# test.py can set this to capture profiling info
LAST_RESULT = None
TRACE = False


def kernel(source, target, weight):
    from concourse.bass_utils import run_bass_kernel_spmd

    global LAST_RESULT
    in_maps = make_host_inputs(
        np.asarray(source), np.asarray(target), np.asarray(weight)
    )
    nc = _get_nc()
    res = run_bass_kernel_spmd(
        nc, in_maps, core_ids=list(range(N_CORES)), trace=TRACE
    )
    LAST_RESULT = res
    outs = [r["out"] for r in res.results]
    full = np.concatenate(outs, axis=0).reshape(B, S, D)
    return np.ascontiguousarray(full.astype(np.float32, copy=False))

